# revision 1
# baseline (speedup 1.0000x reference)
"""MCTC relative-position self-attention on 8 Trainium2 NeuronCores.

Sharding: core = (batch b, head-pair hp): b = core//2, heads {2*hp, 2*hp+1}
of that batch. Each core computes full attention for its 2 heads.

Key trick: rel_pos_rotate(rel)[b,h,i,j] == rel[b,h, M-1+j-i, i], so with
D = q @ E^T of shape [S, L] (L = 2M-1), the rotated matrix is simply
D_flat viewed with row stride L-1 and offset M-1:
    rot[i, j] = D_flat[i*(L-1) + (M-1) + j]
which is a plain strided DMA from a DRAM scratch — no compute.

Matmuls run as float32r (full PE rate at N>=256). Softmax skips the
max-subtraction (scores are O(3), exp is safe in fp32); the 1/sqrt(hd)
scale is folded into the Exp activation's scale; row-sums come from the
activation's accum_out in the same instruction.
"""

import math
import sys

if "/opt/trn_rl_repo" not in sys.path:
    sys.path.insert(0, "/opt/trn_rl_repo")

import numpy as np

import concourse.bass as bass
import concourse.mybir as mybir
import concourse.tile as tile
from concourse import bacc
from concourse.bass_utils import run_bass_kernel_spmd
from concourse.masks import make_identity

S = 920
DMODEL = 1536
HD = 384
M = 920
L = 2 * M - 1  # 1839
NH_PER_CORE = 2

F32 = mybir.dt.float32
# float32r would be 4x faster on the PE but the BIR verifier requires
# producers to pre-round fp32r operands (bitcast alone is rejected).
MM_DT = mybir.dt.float32

P = 128
NS = 8  # ceil(920/128) s-chunks, last has 24 rows
ND = 12  # 1536/128 contraction chunks for projections
NF = 3  # 384/128 feature chunks
NQK = 460  # half of 920, one PSUM bank


def _pc(c):
    return min(P, S - c * P)


def _mm(nc, out, lhsT, rhs, **kw):
    nc.tensor.matmul(out, lhsT.bitcast(MM_DT), rhs.bitcast(MM_DT), **kw)


def build_kernel():
    nc = bacc.Bacc("TRN2", target_bir_lowering=False, debug=False)

    x_d = nc.dram_tensor("x", [S, DMODEL], F32, kind="ExternalInput")
    wq_d = nc.dram_tensor("wq", [DMODEL, NH_PER_CORE * HD], F32, kind="ExternalInput")
    wk_d = nc.dram_tensor("wk", [DMODEL, NH_PER_CORE * HD], F32, kind="ExternalInput")
    wv_d = nc.dram_tensor("wv", [DMODEL, NH_PER_CORE * HD], F32, kind="ExternalInput")
    et_d = nc.dram_tensor("et", [HD, L], F32, kind="ExternalInput")
    out_d = nc.dram_tensor("out", [NH_PER_CORE, S, HD], F32, kind="ExternalOutput")

    from contextlib import ExitStack

    with tile.TileContext(nc) as tc, ExitStack() as ctx:
            ep = ctx.enter_context
            xt_pool = ep(tc.tile_pool(name="xt", bufs=1))
            et_pool = ep(tc.tile_pool(name="et", bufs=1))
            xin_pool = ep(tc.tile_pool(name="xin", bufs=2))
            wch_pool = ep(tc.tile_pool(name="wchunk", bufs=6))
            wv_pool = ep(tc.tile_pool(name="wvres", bufs=1))
            qkt_pool = ep(tc.tile_pool(name="qkt", bufs=1))
            v_pool = ep(tc.tile_pool(name="vsb", bufs=1))
            dst_pool = ep(tc.tile_pool(name="dstage", bufs=3))
            sc_pool = ep(tc.tile_pool(name="sc", bufs=3))
            rel_pool = ep(tc.tile_pool(name="rel", bufs=2))
            pT_pool = ep(tc.tile_pool(name="pT", bufs=1))
            out_pool = ep(tc.tile_pool(name="outp", bufs=2))
            small_pool = ep(tc.tile_pool(name="small", bufs=1))
            pmm = ep(tc.tile_pool(name="pmm", bufs=4, space="PSUM"))
            pv = ep(tc.tile_pool(name="pv", bufs=2, space="PSUM"))
            pt = ep(tc.tile_pool(name="pt", bufs=2, space="PSUM"))
            dram_pool = ep(tc.tile_pool(name="dram", bufs=2, space="DRAM"))

            ident = small_pool.tile([P, P], F32, tag="ident")
            make_identity(nc, ident)

            # ---- load E^T [384, 1839] -> [128, 3, 1839] ----
            et_sb = et_pool.tile([P, NF, L], F32, tag="et")
            et_view = et_d.ap().rearrange("(j p) l -> p j l", p=P)
            for j in range(NF):
                half = L // 2
                nc.sync.dma_start(et_sb[:, j, :half], et_view[:, j, :half])
                nc.sync.dma_start(et_sb[:, j, half:], et_view[:, j, half:])

            # ---- X -> X^T via PE transposes: xt [128, 12, 920] ----
            xt_sb = xt_pool.tile([P, ND, S], F32, tag="xt")
            for c in range(NS):
                pc = _pc(c)
                x_in = xin_pool.tile([P, DMODEL], F32, tag="xin")
                nc.sync.dma_start(
                    x_in[:pc, : DMODEL // 2], x_d.ap()[c * P : c * P + pc, : DMODEL // 2]
                )
                nc.sync.dma_start(
                    x_in[:pc, DMODEL // 2 :], x_d.ap()[c * P : c * P + pc, DMODEL // 2 :]
                )
                for d in range(ND):
                    ps = pt.tile([P, P], F32, tag="pt")
                    nc.tensor.transpose(
                        ps[:P, :pc], x_in[:pc, d * P : (d + 1) * P], ident[:pc, :pc]
                    )
                    nc.vector.tensor_copy(xt_sb[:, d, c * P : c * P + pc], ps[:P, :pc])

            for h in range(NH_PER_CORE):
                hs = h * HD

                # ---- q^T / k^T projections: [384, 920] = W_chunk.T @ X^T ----
                qT_sb = qkt_pool.tile([P, NF, S], F32, tag="qT")
                kT_sb = qkt_pool.tile([P, NF, S], F32, tag="kT")
                for w_d, dst in ((wq_d, qT_sb), (wk_d, kT_sb)):
                    for m in range(NF):
                        ps0 = pmm.tile([P, NQK], F32, tag="pmm")
                        ps1 = pmm.tile([P, NQK], F32, tag="pmm")
                        for kd in range(ND):
                            wch = wch_pool.tile([P, P], F32, tag="wch")
                            nc.sync.dma_start(
                                wch[:],
                                w_d.ap()[
                                    kd * P : (kd + 1) * P, hs + m * P : hs + (m + 1) * P
                                ],
                            )
                            _mm(
                                nc, ps0[:], wch[:], xt_sb[:, kd, :NQK],
                                start=(kd == 0), stop=(kd == ND - 1),
                            )
                            _mm(
                                nc, ps1[:], wch[:], xt_sb[:, kd, NQK:],
                                start=(kd == 0), stop=(kd == ND - 1),
                            )
                        nc.vector.tensor_copy(dst[:, m, :NQK], ps0[:])
                        nc.vector.tensor_copy(dst[:, m, NQK:], ps1[:])

                # ---- v projection (natural layout): [920, 384] ----
                wv_sb = wv_pool.tile([P, ND, HD], F32, tag="wv")
                wv_view = wv_d.ap()[:, hs : hs + HD].rearrange("(j p) f -> p j f", p=P)
                nc.sync.dma_start(wv_sb[:, : ND // 2, :], wv_view[:, : ND // 2, :])
                nc.sync.dma_start(wv_sb[:, ND // 2 :, :], wv_view[:, ND // 2 :, :])
                v_sb = v_pool.tile([P, NS, HD], F32, tag="v")
                for c in range(NS):
                    pc = _pc(c)
                    ps = pv.tile([P, HD], F32, tag="pv")
                    for kd in range(ND):
                        _mm(
                            nc, ps[:pc, :], xt_sb[:, kd, c * P : c * P + pc],
                            wv_sb[:, kd, :],
                            start=(kd == 0), stop=(kd == ND - 1),
                        )
                    nc.vector.tensor_copy(v_sb[:pc, c, :], ps[:pc, :])

                # ---- D = q E^T into DRAM scratch (only needed l-columns) ----
                d_dram = dram_pool.tile([S, L], F32, tag="dscratch")
                d_flat = d_dram.rearrange("a b -> (a b)")
                for c in range(NS):
                    pc = _pc(c)
                    i_max = c * P + pc - 1
                    l_lo = (M - 1) - i_max
                    l_hi = (L - 1) - c * P + 1
                    width = l_hi - l_lo
                    nt = 3
                    base = width // nt
                    sizes = [base + (1 if i < width % nt else 0) for i in range(nt)]
                    off = l_lo
                    for w in sizes:
                        ps = pmm.tile([P, NQK], F32, tag="pmm")
                        for kd in range(NF):
                            _mm(
                                nc, ps[:pc, :w],
                                qT_sb[:, kd, c * P : c * P + pc],
                                et_sb[:, kd, off : off + w],
                                start=(kd == 0), stop=(kd == NF - 1),
                            )
                        dstg = dst_pool.tile([P, NQK], F32, tag="dstg")
                        nc.vector.tensor_copy(dstg[:pc, :w], ps[:pc, :w])
                        nc.sync.dma_start(
                            d_dram[c * P : c * P + pc, off : off + w], dstg[:pc, :w]
                        )
                        off += w

                # ---- scores + rel + exp (+row-sum) per q-chunk ----
                denom = small_pool.tile([P, NS], F32, tag=f"den{h}")
                rden = small_pool.tile([P, NS], F32, tag=f"rden{h}")
                sc_tiles = []
                for c in range(NS):
                    pc = _pc(c)
                    rel_sb = rel_pool.tile([P, S], F32, tag="rel")
                    skew = (
                        d_flat[
                            (M - 1) + c * P * (L - 1) :
                            (M - 1) + c * P * (L - 1) + pc * (L - 1)
                        ]
                        .rearrange("(p x) -> p x", x=L - 1)
                    )
                    nc.sync.dma_start(rel_sb[:pc, :NQK], skew[:, :NQK])
                    nc.sync.dma_start(rel_sb[:pc, NQK:S], skew[:, NQK:S])

                    sc_sb = sc_pool.tile([P, S], F32, tag="sc")
                    for n in range(2):
                        ps = pmm.tile([P, NQK], F32, tag="pmm")
                        for kd in range(NF):
                            _mm(
                                nc, ps[:pc, :],
                                qT_sb[:, kd, c * P : c * P + pc],
                                kT_sb[:, kd, n * NQK : (n + 1) * NQK],
                                start=(kd == 0), stop=(kd == NF - 1),
                            )
                        nc.vector.tensor_add(
                            sc_sb[:pc, n * NQK : (n + 1) * NQK],
                            ps[:pc, :],
                            rel_sb[:pc, n * NQK : (n + 1) * NQK],
                        )
                    nc.scalar.activation(
                        sc_sb[:pc, :],
                        sc_sb[:pc, :],
                        mybir.ActivationFunctionType.Exp,
                        scale=float(1.0 / math.sqrt(HD)),
                        accum_out=denom[:pc, c : c + 1],
                    )
                    nc.vector.reciprocal(rden[:pc, c : c + 1], denom[:pc, c : c + 1])
                    sc_tiles.append(sc_sb)

                # ---- transpose exp(scores) -> probsT [k-part, q] ----
                pT_sb = pT_pool.tile([P, NS, S], F32, tag="pT")
                for c in range(NS):
                    pc = _pc(c)
                    for kc in range(NS):
                        pkc = _pc(kc)
                        ps = pt.tile([P, P], F32, tag="pt")
                        nc.tensor.transpose(
                            ps[:pkc, :pc],
                            sc_tiles[c][:pc, kc * P : kc * P + pkc],
                            ident[:pc, :pc],
                        )
                        nc.vector.tensor_copy(
                            pT_sb[:pkc, kc, c * P : c * P + pc], ps[:pkc, :pc]
                        )

                # ---- ctx = probsT.T @ v, normalized by 1/rowsum ----
                for c in range(NS):
                    pc = _pc(c)
                    ps = pv.tile([P, HD], F32, tag="pv")
                    for kc in range(NS):
                        pkc = _pc(kc)
                        _mm(
                            nc, ps[:pc, :],
                            pT_sb[:pkc, kc, c * P : c * P + pc],
                            v_sb[:pkc, kc, :],
                            start=(kc == 0), stop=(kc == NS - 1),
                        )
                    o_sb = out_pool.tile([P, HD], F32, tag="o")
                    nc.vector.tensor_scalar_mul(
                        o_sb[:pc, :], ps[:pc, :], rden[:pc, c : c + 1]
                    )
                    nc.sync.dma_start(
                        out_d.ap()[h, c * P : c * P + pc, :], o_sb[:pc, :]
                    )

    nc.compile()
    return nc


_NC = None
LAST_RESULTS = None


def kernel(hidden_states, q_w, k_w, v_w, dist_emb):
    global _NC, LAST_RESULTS
    if _NC is None:
        _NC = build_kernel()

    hidden_states = np.asarray(hidden_states, dtype=np.float32)
    q_w = np.asarray(q_w, dtype=np.float32)
    k_w = np.asarray(k_w, dtype=np.float32)
    v_w = np.asarray(v_w, dtype=np.float32)
    dist_emb = np.asarray(dist_emb, dtype=np.float32)

    et = np.ascontiguousarray(dist_emb.T)
    in_maps = []
    for core in range(8):
        b, hp = core // 2, core % 2
        sl = slice(hp * NH_PER_CORE * HD, (hp + 1) * NH_PER_CORE * HD)
        in_maps.append(
            {
                "x": np.ascontiguousarray(hidden_states[b]),
                "wq": np.ascontiguousarray(q_w[:, sl]),
                "wk": np.ascontiguousarray(k_w[:, sl]),
                "wv": np.ascontiguousarray(v_w[:, sl]),
                "et": et,
            }
        )

    res = run_bass_kernel_spmd(_NC, in_maps, core_ids=list(range(8)))
    LAST_RESULTS = res

    B = hidden_states.shape[0]
    out = np.empty((B, S, 4 * HD), np.float32)
    for core in range(8):
        b, hp = core // 2, core % 2
        o = res.results[core]["out"]
        for j in range(NH_PER_CORE):
            h = hp * NH_PER_CORE + j
            out[b, :, h * HD : (h + 1) * HD] = o[j]
    return out



# revision 10
# speedup vs baseline: 2.4749x; 2.4749x over previous
"""MCTC relative-position self-attention on 8 Trainium2 NeuronCores.

Sharding: core = (batch b, head-pair hp): b = core//2, heads {2*hp, 2*hp+1}
of that batch. Each core computes full attention for its 2 heads.

Key trick: rel_pos_rotate(rel)[b,h,i,j] == rel[b,h, M-1+j-i, i], so with
D = q @ E^T of shape [S, L] (L = 2M-1), the rotated matrix is simply
D_flat viewed with row stride L-1 and offset M-1:
    rot[i, j] = D_flat[i*(L-1) + (M-1) + j]
which is a plain strided DMA from a DRAM scratch — no compute.

All matmul operands are bf16 (PE runs 4x faster than fp32; PSUM still
accumulates fp32). The rel term is folded into the scores PSUM group via
an identity matmul instead of a vector add. Softmax skips the
max-subtraction (scores are O(3), exp is safe in fp32); the 1/sqrt(hd)
scale is folded into the Exp activation's scale; row-sums come from the
activation's accum_out in the same instruction.
"""

import math
import sys

if "/opt/trn_rl_repo" not in sys.path:
    sys.path.insert(0, "/opt/trn_rl_repo")

import numpy as np
import ml_dtypes

import concourse.bass as bass
import concourse.mybir as mybir
import concourse.tile as tile
from concourse import bacc
from concourse.bass_utils import run_bass_kernel_spmd
from concourse.masks import make_identity

S = 920
DMODEL = 1536
HD = 384
M = 920
L = 2 * M - 1  # 1839
NH_PER_CORE = 2

F32 = mybir.dt.float32
BF16 = mybir.dt.bfloat16
NPBF16 = ml_dtypes.bfloat16

P = 128
NS = 8  # ceil(920/128) s-chunks, last has 24 rows
ND = 12  # 1536/128 contraction chunks for projections
NF = 3  # 384/128 feature chunks
NQK = 460  # half of 920, one PSUM bank


def _pc(c):
    return min(P, S - c * P)


def build_kernel():
    nc = bacc.Bacc("TRN2", target_bir_lowering=False, debug=False)

    x_d = nc.dram_tensor("x", [S, DMODEL], BF16, kind="ExternalInput")
    wq_d = nc.dram_tensor("wq", [DMODEL, NH_PER_CORE * HD], BF16, kind="ExternalInput")
    wk_d = nc.dram_tensor("wk", [DMODEL, NH_PER_CORE * HD], BF16, kind="ExternalInput")
    wv_d = nc.dram_tensor("wv", [DMODEL, NH_PER_CORE * HD], BF16, kind="ExternalInput")
    et_d = nc.dram_tensor("et", [HD, L], BF16, kind="ExternalInput")
    out_d = nc.dram_tensor("out", [NH_PER_CORE, S, HD], F32, kind="ExternalOutput")

    from contextlib import ExitStack

    with tile.TileContext(nc) as tc, ExitStack() as ctx:
            ep = ctx.enter_context
            xt_pool = ep(tc.tile_pool(name="xt", bufs=1))
            et_pool = ep(tc.tile_pool(name="et", bufs=1))
            xin_pool = ep(tc.tile_pool(name="xin", bufs=2))
            wch_pool = ep(tc.tile_pool(name="wchunk", bufs=6))
            wv_pool = ep(tc.tile_pool(name="wvres", bufs=2))
            qkt_pool = ep(tc.tile_pool(name="qkt", bufs=2))
            v_pool = ep(tc.tile_pool(name="vsb", bufs=2))
            dst_pool = ep(tc.tile_pool(name="dstage", bufs=3))
            sc_pool = ep(tc.tile_pool(name="sc", bufs=3))
            rel_pool = ep(tc.tile_pool(name="rel", bufs=3))
            pT_pool = ep(tc.tile_pool(name="pT", bufs=2))
            out_pool = ep(tc.tile_pool(name="outp", bufs=2))
            small_pool = ep(tc.tile_pool(name="small", bufs=1))
            pmm = ep(tc.tile_pool(name="pmm", bufs=4, space="PSUM"))
            pv = ep(tc.tile_pool(name="pv", bufs=2, space="PSUM"))
            pt = ep(tc.tile_pool(name="pt", bufs=2, space="PSUM"))
            dram_pool = ep(tc.tile_pool(name="dram", bufs=2, space="DRAM"))

            ident = small_pool.tile([P, P], BF16, tag="ident")
            make_identity(nc, ident)

            # ---- load E^T [384, 1839] -> [128, 3, 1839] ----
            et_sb = et_pool.tile([P, NF, L], BF16, tag="et")
            et_view = et_d.ap().rearrange("(j p) l -> p j l", p=P)
            for j in range(NF):
                half = L // 2
                nc.sync.dma_start(et_sb[:, j, :half], et_view[:, j, :half])
                nc.sync.dma_start(et_sb[:, j, half:], et_view[:, j, half:])

            # ---- X -> X^T via PE transposes: xt [128, 12, 920] bf16 ----
            xt_sb = xt_pool.tile([P, ND, S], BF16, tag="xt")
            for c in range(NS):
                pc = _pc(c)
                x_in = xin_pool.tile([P, DMODEL], BF16, tag="xin")
                nc.sync.dma_start(
                    x_in[:pc, : DMODEL // 2], x_d.ap()[c * P : c * P + pc, : DMODEL // 2]
                )
                nc.sync.dma_start(
                    x_in[:pc, DMODEL // 2 :], x_d.ap()[c * P : c * P + pc, DMODEL // 2 :]
                )
                for d in range(ND):
                    ps = pt.tile([P, P], BF16, tag="pt")
                    nc.tensor.transpose(
                        ps[:P, :pc], x_in[:pc, d * P : (d + 1) * P], ident[:pc, :pc]
                    )
                    nc.vector.tensor_copy(xt_sb[:, d, c * P : c * P + pc], ps[:P, :pc])

            for h in range(NH_PER_CORE):
                hs = h * HD

                # ---- q^T / k^T projections: [384, 920] = W_chunk.T @ X^T ----
                qT_sb = qkt_pool.tile([P, NF, S], BF16, tag="qT")
                kT_sb = qkt_pool.tile([P, NF, S], BF16, tag="kT")
                for wi, (w_d, dst) in enumerate(((wq_d, qT_sb), (wk_d, kT_sb))):
                    for m in range(NF):
                        ps0 = pmm.tile([P, NQK], F32, tag="pmm")
                        ps1 = pmm.tile([P, NQK], F32, tag="pmm")
                        for kd in range(ND):
                            wch = wch_pool.tile([P, P], BF16, tag="wch")
                            nc.sync.dma_start(
                                wch[:],
                                w_d.ap()[
                                    kd * P : (kd + 1) * P, hs + m * P : hs + (m + 1) * P
                                ],
                            )
                            nc.tensor.matmul(
                                ps0[:], wch[:], xt_sb[:, kd, :NQK],
                                start=(kd == 0), stop=(kd == ND - 1),
                            )
                            nc.tensor.matmul(
                                ps1[:], wch[:], xt_sb[:, kd, NQK:],
                                start=(kd == 0), stop=(kd == ND - 1),
                            )
                        if wi == 0:
                            nc.scalar.copy(dst[:, m, :NQK], ps0[:])
                            nc.scalar.copy(dst[:, m, NQK:], ps1[:])
                        else:
                            nc.vector.tensor_copy(dst[:, m, :NQK], ps0[:])
                            nc.vector.tensor_copy(dst[:, m, NQK:], ps1[:])

                # ---- v projection (natural layout): [920, 384] ----
                wv_sb = wv_pool.tile([P, ND, HD], BF16, tag="wv")
                wv_view = wv_d.ap()[:, hs : hs + HD].rearrange("(j p) f -> p j f", p=P)
                nc.sync.dma_start(wv_sb[:, : ND // 2, :], wv_view[:, : ND // 2, :])
                nc.sync.dma_start(wv_sb[:, ND // 2 :, :], wv_view[:, ND // 2 :, :])
                v_sb = v_pool.tile([P, NS, HD], BF16, tag="v")
                for c in range(NS):
                    pc = _pc(c)
                    ps = pv.tile([P, HD], F32, tag="pv")
                    for kd in range(ND):
                        nc.tensor.matmul(
                            ps[:pc, :], xt_sb[:, kd, c * P : c * P + pc],
                            wv_sb[:, kd, :],
                            start=(kd == 0), stop=(kd == ND - 1),
                        )
                    nc.scalar.copy(v_sb[:pc, c, :], ps[:pc, :])

                # ---- D = q E^T into DRAM scratch (only needed l-columns) ----
                d_dram = dram_pool.tile([S, L], BF16, tag="dscratch")
                d_flat = d_dram.rearrange("a b -> (a b)")

                def _cp(i, out_ap, in_ap):
                    if i % 2 == 0:
                        nc.vector.tensor_copy(out_ap, in_ap)
                    else:
                        nc.scalar.copy(out_ap, in_ap)
                for c in range(NS):
                    pc = _pc(c)
                    i_max = c * P + pc - 1
                    l_lo = (M - 1) - i_max
                    l_hi = (L - 1) - c * P + 1
                    width = l_hi - l_lo
                    nt = 3
                    base = width // nt
                    sizes = [base + (1 if i < width % nt else 0) for i in range(nt)]
                    off = l_lo
                    for ti, w in enumerate(sizes):
                        ps = pmm.tile([P, NQK], F32, tag="pmm")
                        for kd in range(NF):
                            nc.tensor.matmul(
                                ps[:pc, :w],
                                qT_sb[:, kd, c * P : c * P + pc],
                                et_sb[:, kd, off : off + w],
                                start=(kd == 0), stop=(kd == NF - 1),
                            )
                        dstg = dst_pool.tile([P, NQK], BF16, tag="dstg")
                        _cp(c * nt + ti, dstg[:pc, :w], ps[:pc, :w])
                        nc.sync.dma_start(
                            d_dram[c * P : c * P + pc, off : off + w], dstg[:pc, :w]
                        )
                        off += w

                # ---- per q-chunk: scores + rel + exp, transpose, ctx ----
                den_a = small_pool.tile([P, NS], F32, tag=f"dena{h}")
                den_b = small_pool.tile([P, NS], F32, tag=f"denb{h}")
                rden = small_pool.tile([P, NS], F32, tag=f"rden{h}")
                for c in range(NS):
                    pc = _pc(c)
                    rel_sb = rel_pool.tile([P, S], BF16, tag="rel")
                    skew = (
                        d_flat[
                            (M - 1) + c * P * (L - 1) :
                            (M - 1) + c * P * (L - 1) + pc * (L - 1)
                        ]
                        .rearrange("(p x) -> p x", x=L - 1)
                    )
                    nc.sync.dma_start(rel_sb[:pc, :NQK], skew[:, :NQK])
                    nc.sync.dma_start(rel_sb[:pc, NQK:S], skew[:, NQK:S])

                    sc_sb = sc_pool.tile([P, S], BF16, tag="sc")
                    for n, den in ((0, den_a), (1, den_b)):
                        ps = pmm.tile([P, NQK], F32, tag="pmm")
                        for kd in range(NF):
                            nc.tensor.matmul(
                                ps[:pc, :],
                                qT_sb[:, kd, c * P : c * P + pc],
                                kT_sb[:, kd, n * NQK : (n + 1) * NQK],
                                start=(kd == 0), stop=False,
                            )
                        nc.tensor.matmul(
                            ps[:pc, :],
                            ident[:pc, :pc],
                            rel_sb[:pc, n * NQK : (n + 1) * NQK],
                            start=False, stop=True,
                        )
                        nc.scalar.activation(
                            sc_sb[:pc, n * NQK : (n + 1) * NQK],
                            ps[:pc, :],
                            mybir.ActivationFunctionType.Exp,
                            scale=float(1.0 / math.sqrt(HD)),
                            accum_out=den[:pc, c : c + 1],
                        )

                    # transpose exp(scores) chunk -> pT [k-part, q]
                    pT_sb = pT_pool.tile([P, NS, P], BF16, tag="pT")
                    for kc in range(NS):
                        pkc = _pc(kc)
                        psT = pt.tile([P, P], BF16, tag="pt")
                        nc.tensor.transpose(
                            psT[:pkc, :pc],
                            sc_sb[:pc, kc * P : kc * P + pkc],
                            ident[:pc, :pc],
                        )
                        _cp(kc, pT_sb[:pkc, kc, :pc], psT[:pkc, :pc])

                    # ctx = probsT.T @ v for this q-chunk
                    psc = pv.tile([P, HD], F32, tag="pv")
                    for kc in range(NS):
                        pkc = _pc(kc)
                        nc.tensor.matmul(
                            psc[:pc, :],
                            pT_sb[:pkc, kc, :pc],
                            v_sb[:pkc, kc, :],
                            start=(kc == 0), stop=(kc == NS - 1),
                        )
                    nc.vector.tensor_add(
                        rden[:pc, c : c + 1], den_a[:pc, c : c + 1], den_b[:pc, c : c + 1]
                    )
                    nc.vector.reciprocal(rden[:pc, c : c + 1], rden[:pc, c : c + 1])
                    o_sb = out_pool.tile([P, HD], F32, tag="o")
                    nc.vector.tensor_scalar_mul(
                        o_sb[:pc, :], psc[:pc, :], rden[:pc, c : c + 1]
                    )
                    nc.sync.dma_start(
                        out_d.ap()[h, c * P : c * P + pc, :], o_sb[:pc, :]
                    )

    nc.compile()
    return nc


_NC = None
LAST_RESULTS = None


def kernel(hidden_states, q_w, k_w, v_w, dist_emb):
    global _NC, LAST_RESULTS
    if _NC is None:
        _NC = build_kernel()

    hidden_states = np.asarray(hidden_states, dtype=np.float32)
    q_w = np.asarray(q_w, dtype=np.float32)
    k_w = np.asarray(k_w, dtype=np.float32)
    v_w = np.asarray(v_w, dtype=np.float32)
    dist_emb = np.asarray(dist_emb, dtype=np.float32)

    et = np.ascontiguousarray(dist_emb.T).astype(NPBF16)
    in_maps = []
    for core in range(8):
        b, hp = core // 2, core % 2
        sl = slice(hp * NH_PER_CORE * HD, (hp + 1) * NH_PER_CORE * HD)
        in_maps.append(
            {
                "x": np.ascontiguousarray(hidden_states[b]).astype(NPBF16),
                "wq": np.ascontiguousarray(q_w[:, sl]).astype(NPBF16),
                "wk": np.ascontiguousarray(k_w[:, sl]).astype(NPBF16),
                "wv": np.ascontiguousarray(v_w[:, sl]).astype(NPBF16),
                "et": et,
            }
        )

    res = run_bass_kernel_spmd(_NC, in_maps, core_ids=list(range(8)))
    LAST_RESULTS = res

    B = hidden_states.shape[0]
    out = np.empty((B, S, 4 * HD), np.float32)
    for core in range(8):
        b, hp = core // 2, core % 2
        o = res.results[core]["out"]
        for j in range(NH_PER_CORE):
            h = hp * NH_PER_CORE + j
            out[b, :, h * HD : (h + 1) * HD] = o[j]
    return out


# revision 16
# speedup vs baseline: 2.4752x; 1.0001x over previous
"""MCTC relative-position self-attention on 8 Trainium2 NeuronCores.

Sharding: core = (batch b, head-pair hp): b = core//2, heads {2*hp, 2*hp+1}
of that batch. Each core computes full attention for its 2 heads.

Key trick: rel_pos_rotate(rel)[b,h,i,j] == rel[b,h, M-1+j-i, i], so with
D = q @ E^T of shape [S, L] (L = 2M-1), the rotated matrix is simply
D_flat viewed with row stride L-1 and offset M-1:
    rot[i, j] = D_flat[i*(L-1) + (M-1) + j]
which is a plain strided DMA from a DRAM scratch — no compute.

All matmul operands are bf16 (PE runs 4x faster than fp32; PSUM still
accumulates fp32). The rel term is folded into the scores PSUM group via
an identity matmul instead of a vector add. Softmax skips the
max-subtraction (scores are O(3), exp is safe in fp32); the 1/sqrt(hd)
scale is folded into the Exp activation's scale; row-sums come from the
activation's accum_out in the same instruction.
"""

import math
import sys

if "/opt/trn_rl_repo" not in sys.path:
    sys.path.insert(0, "/opt/trn_rl_repo")

import numpy as np
import ml_dtypes

import concourse.bass as bass
import concourse.mybir as mybir
import concourse.tile as tile
from concourse import bacc
from concourse.bass_utils import run_bass_kernel_spmd
from concourse.masks import make_identity

S = 920
DMODEL = 1536
HD = 384
M = 920
L = 2 * M - 1  # 1839
NH_PER_CORE = 2

F32 = mybir.dt.float32
BF16 = mybir.dt.bfloat16
NPBF16 = ml_dtypes.bfloat16

P = 128
NS = 8  # ceil(920/128) s-chunks, last has 24 rows
ND = 12  # 1536/128 contraction chunks for projections
NF = 3  # 384/128 feature chunks
NQK = 460  # half of 920, one PSUM bank


def _pc(c):
    return min(P, S - c * P)


def build_kernel():
    nc = bacc.Bacc("TRN2", target_bir_lowering=False, debug=False)

    xt_d = nc.dram_tensor("xt", [DMODEL, S], BF16, kind="ExternalInput")
    wq_d = nc.dram_tensor("wq", [DMODEL, NH_PER_CORE * HD], BF16, kind="ExternalInput")
    wk_d = nc.dram_tensor("wk", [DMODEL, NH_PER_CORE * HD], BF16, kind="ExternalInput")
    wv_d = nc.dram_tensor("wv", [DMODEL, NH_PER_CORE * HD], BF16, kind="ExternalInput")
    et_d = nc.dram_tensor("et", [HD, L], BF16, kind="ExternalInput")
    out_d = nc.dram_tensor("out", [NH_PER_CORE, S, HD], F32, kind="ExternalOutput")

    from contextlib import ExitStack

    with tile.TileContext(nc) as tc, ExitStack() as ctx:
            ep = ctx.enter_context
            xt_pool = ep(tc.tile_pool(name="xt", bufs=1))
            et_pool = ep(tc.tile_pool(name="et", bufs=1))
            wqk_pool = ep(tc.tile_pool(name="wqk", bufs=2))
            wv_pool = ep(tc.tile_pool(name="wvres", bufs=2))
            qkt_pool = ep(tc.tile_pool(name="qkt", bufs=2))
            v_pool = ep(tc.tile_pool(name="vsb", bufs=2))
            dst_pool = ep(tc.tile_pool(name="dstage", bufs=6))
            sc_pool = ep(tc.tile_pool(name="sc", bufs=3))
            rel_pool = ep(tc.tile_pool(name="rel", bufs=3))
            pT_pool = ep(tc.tile_pool(name="pT", bufs=2))
            out_pool = ep(tc.tile_pool(name="outp", bufs=2))
            small_pool = ep(tc.tile_pool(name="small", bufs=1))
            pmm = ep(tc.tile_pool(name="pmm", bufs=4, space="PSUM"))
            pv = ep(tc.tile_pool(name="pv", bufs=2, space="PSUM"))
            pt = ep(tc.tile_pool(name="pt", bufs=2, space="PSUM"))
            dram_pool = ep(tc.tile_pool(name="dram", bufs=2, space="DRAM"))

            ident = small_pool.tile([P, P], BF16, tag="ident")
            make_identity(nc, ident)

            # ---- load E^T [384, 1839] -> [128, 3, 1839] ----
            et_sb = et_pool.tile([P, NF, L], BF16, tag="et")
            et_view = et_d.ap().rearrange("(j p) l -> p j l", p=P)
            for j in range(NF):
                half = L // 2
                nc.sync.dma_start(et_sb[:, j, :half], et_view[:, j, :half])
                nc.sync.dma_start(et_sb[:, j, half:], et_view[:, j, half:])

            # ---- X^T pre-transposed on host: xt [128, 12, 920] bf16 ----
            xt_sb = xt_pool.tile([P, ND, S], BF16, tag="xt")
            xt_view = xt_d.ap().rearrange("(j p) s -> p j s", p=P)
            for j in range(0, ND, 3):
                nc.sync.dma_start(xt_sb[:, j : j + 3, :], xt_view[:, j : j + 3, :])

            for h in range(NH_PER_CORE):
                hs = h * HD

                # ---- q^T / k^T projections: [384, 920] = W_chunk.T @ X^T ----
                qT_sb = qkt_pool.tile([P, NF, S], BF16, tag="qT")
                kT_sb = qkt_pool.tile([P, NF, S], BF16, tag="kT")
                for wi, (w_d, dst) in enumerate(((wq_d, qT_sb), (wk_d, kT_sb))):
                    w_sb = wqk_pool.tile([P, ND, HD], BF16, tag=f"wqk{wi}")
                    w_view = w_d.ap()[:, hs : hs + HD].rearrange(
                        "(j p) f -> p j f", p=P
                    )
                    nc.sync.dma_start(w_sb[:, : ND // 2, :], w_view[:, : ND // 2, :])
                    nc.sync.dma_start(w_sb[:, ND // 2 :, :], w_view[:, ND // 2 :, :])
                    for m in range(NF):
                        ps0 = pmm.tile([P, NQK], F32, tag="pmm")
                        ps1 = pmm.tile([P, NQK], F32, tag="pmm")
                        for kd in range(ND):
                            wch = w_sb[:, kd, m * P : (m + 1) * P]
                            nc.tensor.matmul(
                                ps0[:], wch, xt_sb[:, kd, :NQK],
                                start=(kd == 0), stop=(kd == ND - 1),
                            )
                            nc.tensor.matmul(
                                ps1[:], wch, xt_sb[:, kd, NQK:],
                                start=(kd == 0), stop=(kd == ND - 1),
                            )
                        if wi == 0:
                            nc.scalar.copy(dst[:, m, :NQK], ps0[:])
                            nc.scalar.copy(dst[:, m, NQK:], ps1[:])
                        else:
                            nc.vector.tensor_copy(dst[:, m, :NQK], ps0[:])
                            nc.vector.tensor_copy(dst[:, m, NQK:], ps1[:])

                # ---- v projection (natural layout): [920, 384] ----
                wv_sb = wv_pool.tile([P, ND, HD], BF16, tag="wv")
                wv_view = wv_d.ap()[:, hs : hs + HD].rearrange("(j p) f -> p j f", p=P)
                nc.sync.dma_start(wv_sb[:, : ND // 2, :], wv_view[:, : ND // 2, :])
                nc.sync.dma_start(wv_sb[:, ND // 2 :, :], wv_view[:, ND // 2 :, :])
                v_sb = v_pool.tile([P, NS, HD], BF16, tag="v")
                for c in range(NS):
                    pc = _pc(c)
                    ps = pv.tile([P, HD], F32, tag="pv")
                    for kd in range(ND):
                        nc.tensor.matmul(
                            ps[:pc, :], xt_sb[:, kd, c * P : c * P + pc],
                            wv_sb[:, kd, :],
                            start=(kd == 0), stop=(kd == ND - 1),
                        )
                    nc.scalar.copy(v_sb[:pc, c, :], ps[:pc, :])

                # ---- D = q E^T into DRAM scratch (only needed l-columns) ----
                d_dram = dram_pool.tile([S, L], BF16, tag="dscratch")
                d_flat = d_dram.rearrange("a b -> (a b)")

                def _cp(i, out_ap, in_ap):
                    if i % 2 == 0:
                        nc.vector.tensor_copy(out_ap, in_ap)
                    else:
                        nc.scalar.copy(out_ap, in_ap)
                for c in range(NS):
                    pc = _pc(c)
                    i_max = c * P + pc - 1
                    l_lo = (M - 1) - i_max
                    l_hi = (L - 1) - c * P + 1
                    width = l_hi - l_lo
                    nt = 3
                    base = width // nt
                    sizes = [base + (1 if i < width % nt else 0) for i in range(nt)]
                    off = l_lo
                    for ti, w in enumerate(sizes):
                        ps = pmm.tile([P, NQK], F32, tag="pmm")
                        for kd in range(NF):
                            nc.tensor.matmul(
                                ps[:pc, :w],
                                qT_sb[:, kd, c * P : c * P + pc],
                                et_sb[:, kd, off : off + w],
                                start=(kd == 0), stop=(kd == NF - 1),
                            )
                        dstg = dst_pool.tile([P, NQK], BF16, tag="dstg")
                        _cp(c * nt + ti, dstg[:pc, :w], ps[:pc, :w])
                        nc.sync.dma_start(
                            d_dram[c * P : c * P + pc, off : off + w], dstg[:pc, :w]
                        )
                        off += w

                # ---- per q-chunk: scores + rel + exp, transpose, ctx ----
                den_a = small_pool.tile([P, NS], F32, tag=f"dena{h}")
                den_b = small_pool.tile([P, NS], F32, tag=f"denb{h}")
                rden = small_pool.tile([P, NS], F32, tag=f"rden{h}")
                for c in range(NS):
                    pc = _pc(c)
                    rel_sb = rel_pool.tile([P, S], BF16, tag="rel")
                    skew = (
                        d_flat[
                            (M - 1) + c * P * (L - 1) :
                            (M - 1) + c * P * (L - 1) + pc * (L - 1)
                        ]
                        .rearrange("(p x) -> p x", x=L - 1)
                    )
                    nc.sync.dma_start(rel_sb[:pc, :NQK], skew[:, :NQK])
                    nc.sync.dma_start(rel_sb[:pc, NQK:S], skew[:, NQK:S])

                    sc_sb = sc_pool.tile([P, S], BF16, tag="sc")
                    for n, den in ((0, den_a), (1, den_b)):
                        ps = pmm.tile([P, NQK], F32, tag="pmm")
                        for kd in range(NF):
                            nc.tensor.matmul(
                                ps[:pc, :],
                                qT_sb[:, kd, c * P : c * P + pc],
                                kT_sb[:, kd, n * NQK : (n + 1) * NQK],
                                start=(kd == 0), stop=False,
                            )
                        nc.tensor.matmul(
                            ps[:pc, :],
                            ident[:pc, :pc],
                            rel_sb[:pc, n * NQK : (n + 1) * NQK],
                            start=False, stop=True,
                        )
                        nc.scalar.activation(
                            sc_sb[:pc, n * NQK : (n + 1) * NQK],
                            ps[:pc, :],
                            mybir.ActivationFunctionType.Exp,
                            scale=float(1.0 / math.sqrt(HD)),
                            accum_out=den[:pc, c : c + 1],
                        )

                    # transpose exp(scores) chunk -> pT [k-part, q]
                    pT_sb = pT_pool.tile([P, NS, P], BF16, tag="pT")
                    for kc in range(NS):
                        pkc = _pc(kc)
                        psT = pt.tile([P, P], BF16, tag="pt")
                        nc.tensor.transpose(
                            psT[:pkc, :pc],
                            sc_sb[:pc, kc * P : kc * P + pkc],
                            ident[:pc, :pc],
                        )
                        _cp(kc, pT_sb[:pkc, kc, :pc], psT[:pkc, :pc])

                    # ctx = probsT.T @ v for this q-chunk
                    psc = pv.tile([P, HD], F32, tag="pv")
                    for kc in range(NS):
                        pkc = _pc(kc)
                        nc.tensor.matmul(
                            psc[:pc, :],
                            pT_sb[:pkc, kc, :pc],
                            v_sb[:pkc, kc, :],
                            start=(kc == 0), stop=(kc == NS - 1),
                        )
                    nc.vector.tensor_add(
                        rden[:pc, c : c + 1], den_a[:pc, c : c + 1], den_b[:pc, c : c + 1]
                    )
                    nc.vector.reciprocal(rden[:pc, c : c + 1], rden[:pc, c : c + 1])
                    o_sb = out_pool.tile([P, HD], F32, tag="o")
                    nc.vector.tensor_scalar_mul(
                        o_sb[:pc, :], psc[:pc, :], rden[:pc, c : c + 1]
                    )
                    nc.sync.dma_start(
                        out_d.ap()[h, c * P : c * P + pc, :], o_sb[:pc, :]
                    )

    nc.compile()
    return nc


_NC = None
LAST_RESULTS = None


def kernel(hidden_states, q_w, k_w, v_w, dist_emb):
    global _NC, LAST_RESULTS
    if _NC is None:
        _NC = build_kernel()

    hidden_states = np.asarray(hidden_states, dtype=np.float32)
    q_w = np.asarray(q_w, dtype=np.float32)
    k_w = np.asarray(k_w, dtype=np.float32)
    v_w = np.asarray(v_w, dtype=np.float32)
    dist_emb = np.asarray(dist_emb, dtype=np.float32)

    et = np.ascontiguousarray(dist_emb.T).astype(NPBF16)
    in_maps = []
    for core in range(8):
        b, hp = core // 2, core % 2
        sl = slice(hp * NH_PER_CORE * HD, (hp + 1) * NH_PER_CORE * HD)
        in_maps.append(
            {
                "xt": np.ascontiguousarray(hidden_states[b].T).astype(NPBF16),
                "wq": np.ascontiguousarray(q_w[:, sl]).astype(NPBF16),
                "wk": np.ascontiguousarray(k_w[:, sl]).astype(NPBF16),
                "wv": np.ascontiguousarray(v_w[:, sl]).astype(NPBF16),
                "et": et,
            }
        )

    res = run_bass_kernel_spmd(_NC, in_maps, core_ids=list(range(8)))
    LAST_RESULTS = res

    B = hidden_states.shape[0]
    out = np.empty((B, S, 4 * HD), np.float32)
    for core in range(8):
        b, hp = core // 2, core % 2
        o = res.results[core]["out"]
        for j in range(NH_PER_CORE):
            h = hp * NH_PER_CORE + j
            out[b, :, h * HD : (h + 1) * HD] = o[j]
    return out


# revision 17
# speedup vs baseline: 3.0234x; 1.2215x over previous
"""MCTC relative-position self-attention on 8 Trainium2 NeuronCores.

Sharding: core = (batch b, head-pair hp): b = core//2, heads {2*hp, 2*hp+1}
of that batch. Each core computes full attention for its 2 heads.

Key trick: rel_pos_rotate(rel)[b,h,i,j] == rel[b,h, M-1+j-i, i], so with
D = q @ E^T of shape [S, L] (L = 2M-1), the rotated matrix is simply
D_flat viewed with row stride L-1 and offset M-1:
    rot[i, j] = D_flat[i*(L-1) + (M-1) + j]
which is a plain strided DMA from a DRAM scratch — no compute.

All matmul operands are bf16 (PE runs 4x faster than fp32; PSUM still
accumulates fp32). X is transposed on the host so no PE transposes are
needed for X^T. All weights are preloaded up front in a few large DMAs.
The skewed rel reads are prefetched inside the D loop so the scores
pipeline never waits on the DRAM round trip. Softmax skips the
max-subtraction (scores are O(3), exp is safe in fp32); the 1/sqrt(hd)
scale is folded into the Exp activation's scale; row-sums come from the
activation's accum_out in the same instruction.
"""

import math
import sys

if "/opt/trn_rl_repo" not in sys.path:
    sys.path.insert(0, "/opt/trn_rl_repo")

import numpy as np
import ml_dtypes

import concourse.bass as bass
import concourse.mybir as mybir
import concourse.tile as tile
from concourse import bacc
from concourse.bass_utils import run_bass_kernel_spmd
from concourse.masks import make_identity

S = 920
DMODEL = 1536
HD = 384
M = 920
L = 2 * M - 1  # 1839
NH_PER_CORE = 2

F32 = mybir.dt.float32
BF16 = mybir.dt.bfloat16
NPBF16 = ml_dtypes.bfloat16

P = 128
NS = 8  # ceil(920/128) s-chunks, last has 24 rows
ND = 12  # 1536/128 contraction chunks for projections
NF = 3  # 384/128 feature chunks
NQK = 460  # half of 920, one PSUM bank

# runt chunk (24 rows) first so the kernel doesn't end on its latency tail
CHUNKS = [7, 0, 1, 2, 3, 4, 5, 6]


def _pc(c):
    return min(P, S - c * P)


def build_kernel():
    nc = bacc.Bacc("TRN2", target_bir_lowering=False, debug=False)

    xt_d = nc.dram_tensor("xt", [DMODEL, S], BF16, kind="ExternalInput")
    wq_d = nc.dram_tensor("wq", [DMODEL, NH_PER_CORE * HD], BF16, kind="ExternalInput")
    wk_d = nc.dram_tensor("wk", [DMODEL, NH_PER_CORE * HD], BF16, kind="ExternalInput")
    wv_d = nc.dram_tensor("wv", [DMODEL, NH_PER_CORE * HD], BF16, kind="ExternalInput")
    et_d = nc.dram_tensor("et", [HD, L], BF16, kind="ExternalInput")
    out_d = nc.dram_tensor("out", [NH_PER_CORE, S, HD], F32, kind="ExternalOutput")

    from contextlib import ExitStack

    with tile.TileContext(nc) as tc, ExitStack() as ctx:
            ep = ctx.enter_context
            xt_pool = ep(tc.tile_pool(name="xt", bufs=1))
            et_pool = ep(tc.tile_pool(name="et", bufs=1))
            wqk_pool = ep(tc.tile_pool(name="wqk", bufs=2))
            wv_pool = ep(tc.tile_pool(name="wvres", bufs=2))
            qkt_pool = ep(tc.tile_pool(name="qkt", bufs=2))
            v_pool = ep(tc.tile_pool(name="vsb", bufs=2))
            dst_pool = ep(tc.tile_pool(name="dstage", bufs=6))
            sc32_pool = ep(tc.tile_pool(name="sc32", bufs=2))
            sc_pool = ep(tc.tile_pool(name="sc", bufs=3))
            rel_pool = ep(tc.tile_pool(name="rel", bufs=6))
            pT_pool = ep(tc.tile_pool(name="pT", bufs=2))
            out_pool = ep(tc.tile_pool(name="outp", bufs=2))
            small_pool = ep(tc.tile_pool(name="small", bufs=1))
            pmm = ep(tc.tile_pool(name="pmm", bufs=4, space="PSUM"))
            pv = ep(tc.tile_pool(name="pv", bufs=2, space="PSUM"))
            pt = ep(tc.tile_pool(name="pt", bufs=2, space="PSUM"))
            dram_pool = ep(tc.tile_pool(name="dram", bufs=2, space="DRAM"))

            ident = small_pool.tile([P, P], BF16, tag="ident")
            make_identity(nc, ident)

            # ---- X^T pre-transposed on host: xt [128, 12, 920] bf16 ----
            xt_sb = xt_pool.tile([P, ND, S], BF16, tag="xt")
            xt_view = xt_d.ap().rearrange("(j p) s -> p j s", p=P)
            for j in range(0, ND, 3):
                nc.sync.dma_start(xt_sb[:, j : j + 3, :], xt_view[:, j : j + 3, :])

            # ---- preload all per-head weights in large DMAs ----
            w_tiles = []
            for h in range(NH_PER_CORE):
                hs = h * HD
                tiles = {}
                for wname, w_d in (("wq", wq_d), ("wk", wk_d), ("wv", wv_d)):
                    pool = wv_pool if wname == "wv" else wqk_pool
                    w_sb = pool.tile([P, ND, HD], BF16, tag=wname, name=f"{wname}{h}")
                    w_view = w_d.ap()[:, hs : hs + HD].rearrange(
                        "(j p) f -> p j f", p=P
                    )
                    nc.sync.dma_start(w_sb[:, : ND // 2, :], w_view[:, : ND // 2, :])
                    nc.sync.dma_start(w_sb[:, ND // 2 :, :], w_view[:, ND // 2 :, :])
                    tiles[wname] = w_sb
                w_tiles.append(tiles)

            # ---- load E^T [384, 1839] -> [128, 3, 1839] (needed at D phase) ----
            et_sb = et_pool.tile([P, NF, L], BF16, tag="et")
            et_view = et_d.ap().rearrange("(j p) l -> p j l", p=P)
            for j in range(NF):
                half = L // 2
                nc.sync.dma_start(et_sb[:, j, :half], et_view[:, j, :half])
                nc.sync.dma_start(et_sb[:, j, half:], et_view[:, j, half:])

            def _cp(i, out_ap, in_ap):
                if i % 2 == 0:
                    nc.vector.tensor_copy(out_ap, in_ap)
                else:
                    nc.scalar.copy(out_ap, in_ap)

            for h in range(NH_PER_CORE):
                # ---- q^T / k^T projections: [384, 920] = W_chunk.T @ X^T ----
                qT_sb = qkt_pool.tile([P, NF, S], BF16, tag="qT")
                kT_sb = qkt_pool.tile([P, NF, S], BF16, tag="kT")
                for wi, (wname, dst) in enumerate((("wq", qT_sb), ("wk", kT_sb))):
                    w_sb = w_tiles[h][wname]
                    for m in range(NF):
                        ps0 = pmm.tile([P, NQK], F32, tag="pmm")
                        ps1 = pmm.tile([P, NQK], F32, tag="pmm")
                        for kd in range(ND):
                            wch = w_sb[:, kd, m * P : (m + 1) * P]
                            nc.tensor.matmul(
                                ps0[:], wch, xt_sb[:, kd, :NQK],
                                start=(kd == 0), stop=(kd == ND - 1),
                            )
                            nc.tensor.matmul(
                                ps1[:], wch, xt_sb[:, kd, NQK:],
                                start=(kd == 0), stop=(kd == ND - 1),
                            )
                        if wi == 0:
                            nc.scalar.copy(dst[:, m, :NQK], ps0[:])
                            nc.scalar.copy(dst[:, m, NQK:], ps1[:])
                        else:
                            nc.vector.tensor_copy(dst[:, m, :NQK], ps0[:])
                            nc.vector.tensor_copy(dst[:, m, NQK:], ps1[:])

                # ---- v projection (natural layout): [920, 384] ----
                wv_sb = w_tiles[h]["wv"]
                v_sb = v_pool.tile([P, NS, HD], BF16, tag="v")
                for c in range(NS):
                    pc = _pc(c)
                    ps = pv.tile([P, HD], F32, tag="pv")
                    for kd in range(ND):
                        nc.tensor.matmul(
                            ps[:pc, :], xt_sb[:, kd, c * P : c * P + pc],
                            wv_sb[:, kd, :],
                            start=(kd == 0), stop=(kd == ND - 1),
                        )
                    nc.scalar.copy(v_sb[:pc, c, :], ps[:pc, :])

                # ---- D = q E^T into DRAM scratch; prefetch skewed rel reads ----
                d_dram = dram_pool.tile([S, L], BF16, tag="dscratch")
                d_flat = d_dram.rearrange("a b -> (a b)")
                rel_tiles = {}
                ncp = 0
                for c in CHUNKS:
                    pc = _pc(c)
                    i_max = c * P + pc - 1
                    l_lo = (M - 1) - i_max
                    l_hi = (L - 1) - c * P + 1
                    width = l_hi - l_lo
                    nt = 3
                    base = width // nt
                    sizes = [base + (1 if i < width % nt else 0) for i in range(nt)]
                    off = l_lo
                    for w in sizes:
                        ps = pmm.tile([P, NQK], F32, tag="pmm")
                        for kd in range(NF):
                            nc.tensor.matmul(
                                ps[:pc, :w],
                                qT_sb[:, kd, c * P : c * P + pc],
                                et_sb[:, kd, off : off + w],
                                start=(kd == 0), stop=(kd == NF - 1),
                            )
                        dstg = dst_pool.tile([P, NQK], BF16, tag="dstg")
                        _cp(ncp, dstg[:pc, :w], ps[:pc, :w])
                        ncp += 1
                        nc.sync.dma_start(
                            d_dram[c * P : c * P + pc, off : off + w], dstg[:pc, :w]
                        )
                        off += w
                    # prefetch this chunk's rotated rel rows (skewed view)
                    rel_sb = rel_pool.tile([P, S], BF16, tag="rel")
                    skew = (
                        d_flat[
                            (M - 1) + c * P * (L - 1) :
                            (M - 1) + c * P * (L - 1) + pc * (L - 1)
                        ]
                        .rearrange("(p x) -> p x", x=L - 1)
                    )
                    nc.sync.dma_start(rel_sb[:pc, :NQK], skew[:, :NQK])
                    nc.sync.dma_start(rel_sb[:pc, NQK:S], skew[:, NQK:S])
                    rel_tiles[c] = rel_sb

                # ---- per q-chunk: scores + rel + exp, transpose, ctx ----
                den = small_pool.tile([P, NS], F32, tag=f"den{h}")
                rden = small_pool.tile([P, NS], F32, tag=f"rden{h}")
                for c in CHUNKS:
                    pc = _pc(c)
                    rel_sb = rel_tiles[c]
                    sc32 = sc32_pool.tile([P, S], F32, tag="sc32")
                    sc_sb = sc_pool.tile([P, S], BF16, tag="sc")
                    for n in range(2):
                        ps = pmm.tile([P, NQK], F32, tag="pmm")
                        for kd in range(NF):
                            nc.tensor.matmul(
                                ps[:pc, :],
                                qT_sb[:, kd, c * P : c * P + pc],
                                kT_sb[:, kd, n * NQK : (n + 1) * NQK],
                                start=(kd == 0), stop=(kd == NF - 1),
                            )
                        nc.vector.tensor_add(
                            sc32[:pc, n * NQK : (n + 1) * NQK],
                            ps[:pc, :],
                            rel_sb[:pc, n * NQK : (n + 1) * NQK],
                        )
                    nc.scalar.activation(
                        sc_sb[:pc, :],
                        sc32[:pc, :],
                        mybir.ActivationFunctionType.Exp,
                        scale=float(1.0 / math.sqrt(HD)),
                        accum_out=den[:pc, c : c + 1],
                    )
                    nc.vector.reciprocal(rden[:pc, c : c + 1], den[:pc, c : c + 1])

                    # transpose exp(scores) chunk -> pT [k-part, q]
                    pT_sb = pT_pool.tile([P, NS, P], BF16, tag="pT")
                    for kc in range(NS):
                        pkc = _pc(kc)
                        psT = pt.tile([P, P], BF16, tag="pt")
                        nc.tensor.transpose(
                            psT[:pkc, :pc],
                            sc_sb[:pc, kc * P : kc * P + pkc],
                            ident[:pc, :pc],
                        )
                        _cp(kc, pT_sb[:pkc, kc, :pc], psT[:pkc, :pc])

                    # ctx = probsT.T @ v for this q-chunk
                    psc = pv.tile([P, HD], F32, tag="pv")
                    for kc in range(NS):
                        pkc = _pc(kc)
                        nc.tensor.matmul(
                            psc[:pc, :],
                            pT_sb[:pkc, kc, :pc],
                            v_sb[:pkc, kc, :],
                            start=(kc == 0), stop=(kc == NS - 1),
                        )
                    o_sb = out_pool.tile([P, HD], F32, tag="o")
                    nc.scalar.activation(
                        o_sb[:pc, :],
                        psc[:pc, :],
                        mybir.ActivationFunctionType.Copy,
                        scale=rden[:pc, c : c + 1],
                    )
                    nc.sync.dma_start(
                        out_d.ap()[h, c * P : c * P + pc, :], o_sb[:pc, :]
                    )

    nc.compile()
    return nc


_NC = None
LAST_RESULTS = None


def kernel(hidden_states, q_w, k_w, v_w, dist_emb):
    global _NC, LAST_RESULTS
    if _NC is None:
        _NC = build_kernel()

    hidden_states = np.asarray(hidden_states, dtype=np.float32)
    q_w = np.asarray(q_w, dtype=np.float32)
    k_w = np.asarray(k_w, dtype=np.float32)
    v_w = np.asarray(v_w, dtype=np.float32)
    dist_emb = np.asarray(dist_emb, dtype=np.float32)

    et = np.ascontiguousarray(dist_emb.T).astype(NPBF16)
    in_maps = []
    for core in range(8):
        b, hp = core // 2, core % 2
        sl = slice(hp * NH_PER_CORE * HD, (hp + 1) * NH_PER_CORE * HD)
        in_maps.append(
            {
                "xt": np.ascontiguousarray(hidden_states[b].T).astype(NPBF16),
                "wq": np.ascontiguousarray(q_w[:, sl]).astype(NPBF16),
                "wk": np.ascontiguousarray(k_w[:, sl]).astype(NPBF16),
                "wv": np.ascontiguousarray(v_w[:, sl]).astype(NPBF16),
                "et": et,
            }
        )

    res = run_bass_kernel_spmd(_NC, in_maps, core_ids=list(range(8)))
    LAST_RESULTS = res

    B = hidden_states.shape[0]
    out = np.empty((B, S, 4 * HD), np.float32)
    for core in range(8):
        b, hp = core // 2, core % 2
        o = res.results[core]["out"]
        for j in range(NH_PER_CORE):
            h = hp * NH_PER_CORE + j
            out[b, :, h * HD : (h + 1) * HD] = o[j]
    return out


# revision 20
# speedup vs baseline: 3.0556x; 1.0106x over previous
"""MCTC relative-position self-attention on 8 Trainium2 NeuronCores.

Sharding: core = (batch b, head-pair hp): b = core//2, heads {2*hp, 2*hp+1}
of that batch. Each core computes full attention for its 2 heads.

Key trick: rel_pos_rotate(rel)[b,h,i,j] == rel[b,h, M-1+j-i, i], so with
D = q @ E^T of shape [S, L] (L = 2M-1), the rotated matrix is simply
D_flat viewed with row stride L-1 and offset M-1:
    rot[i, j] = D_flat[i*(L-1) + (M-1) + j]
which is a plain strided DMA from a DRAM scratch — no compute.

All matmul operands are bf16 (PE runs 4x faster than fp32; PSUM still
accumulates fp32). X is transposed on the host so no PE transposes are
needed for X^T. All weights are preloaded up front in a few large DMAs.
The skewed rel reads are prefetched inside the D loop so the scores
pipeline never waits on the DRAM round trip. Softmax skips the
max-subtraction (scores are O(3), exp is safe in fp32); the 1/sqrt(hd)
scale is folded into the Exp activation's scale; row-sums come from the
activation's accum_out in the same instruction.
"""

import math
import sys

if "/opt/trn_rl_repo" not in sys.path:
    sys.path.insert(0, "/opt/trn_rl_repo")

import numpy as np
import ml_dtypes

import concourse.bass as bass
import concourse.mybir as mybir
import concourse.tile as tile
from concourse import bacc
from concourse.bass_utils import run_bass_kernel_spmd
from concourse.masks import make_identity

S = 920
DMODEL = 1536
HD = 384
M = 920
L = 2 * M - 1  # 1839
NH_PER_CORE = 2

F32 = mybir.dt.float32
BF16 = mybir.dt.bfloat16
NPBF16 = ml_dtypes.bfloat16

P = 128
NS = 8  # ceil(920/128) s-chunks, last has 24 rows
ND = 12  # 1536/128 contraction chunks for projections
NF = 3  # 384/128 feature chunks
NQK = 460  # half of 920, one PSUM bank

# runt chunk (24 rows) first so the kernel doesn't end on its latency tail
CHUNKS = [7, 0, 1, 2, 3, 4, 5, 6]


def _pc(c):
    return min(P, S - c * P)


def build_kernel():
    nc = bacc.Bacc("TRN2", target_bir_lowering=False, debug=False)

    xt_d = nc.dram_tensor("xt", [DMODEL, S], BF16, kind="ExternalInput")
    wq_d = nc.dram_tensor("wq", [DMODEL, NH_PER_CORE * HD], BF16, kind="ExternalInput")
    wk_d = nc.dram_tensor("wk", [DMODEL, NH_PER_CORE * HD], BF16, kind="ExternalInput")
    wv_d = nc.dram_tensor("wv", [DMODEL, NH_PER_CORE * HD], BF16, kind="ExternalInput")
    et_d = nc.dram_tensor("et", [HD, L], BF16, kind="ExternalInput")
    out_d = nc.dram_tensor("out", [NH_PER_CORE, S, HD], F32, kind="ExternalOutput")

    from contextlib import ExitStack

    with tile.TileContext(nc) as tc, ExitStack() as ctx:
            ep = ctx.enter_context
            xt_pool = ep(tc.tile_pool(name="xt", bufs=1))
            et_pool = ep(tc.tile_pool(name="et", bufs=1))
            wqk_pool = ep(tc.tile_pool(name="wqk", bufs=2))
            wv_pool = ep(tc.tile_pool(name="wvres", bufs=2))
            qkt_pool = ep(tc.tile_pool(name="qkt", bufs=2))
            v_pool = ep(tc.tile_pool(name="vsb", bufs=2))
            dst_pool = ep(tc.tile_pool(name="dstage", bufs=6))
            sc32_pool = ep(tc.tile_pool(name="sc32", bufs=2))
            sc_pool = ep(tc.tile_pool(name="sc", bufs=3))
            rel_pool = ep(tc.tile_pool(name="rel", bufs=6))
            pT_pool = ep(tc.tile_pool(name="pT", bufs=2))
            out_pool = ep(tc.tile_pool(name="outp", bufs=2))
            small_pool = ep(tc.tile_pool(name="small", bufs=1))
            pmm = ep(tc.tile_pool(name="pmm", bufs=4, space="PSUM"))
            pv = ep(tc.tile_pool(name="pv", bufs=2, space="PSUM"))
            pt = ep(tc.tile_pool(name="pt", bufs=2, space="PSUM"))
            dram_pool = ep(tc.tile_pool(name="dram", bufs=2, space="DRAM"))

            ident = small_pool.tile([P, P], BF16, tag="ident")
            make_identity(nc, ident)

            # ---- X^T pre-transposed on host: xt [128, 12, 920] bf16 ----
            xt_sb = xt_pool.tile([P, ND, S], BF16, tag="xt")
            xt_view = xt_d.ap().rearrange("(j p) s -> p j s", p=P)
            for j in range(0, ND, 3):
                nc.sync.dma_start(xt_sb[:, j : j + 3, :], xt_view[:, j : j + 3, :])

            # ---- preload all per-head weights in large DMAs ----
            w_tiles = []
            for h in range(NH_PER_CORE):
                hs = h * HD
                tiles = {}
                for wname, w_d in (("wq", wq_d), ("wk", wk_d), ("wv", wv_d)):
                    pool = wv_pool if wname == "wv" else wqk_pool
                    w_sb = pool.tile([P, ND, HD], BF16, tag=wname, name=f"{wname}{h}")
                    w_view = w_d.ap()[:, hs : hs + HD].rearrange(
                        "(j p) f -> p j f", p=P
                    )
                    nc.sync.dma_start(w_sb[:, : ND // 2, :], w_view[:, : ND // 2, :])
                    nc.sync.dma_start(w_sb[:, ND // 2 :, :], w_view[:, ND // 2 :, :])
                    tiles[wname] = w_sb
                w_tiles.append(tiles)

            # ---- load E^T [384, 1839] -> [128, 3, 1839] (needed at D phase) ----
            et_sb = et_pool.tile([P, NF, L], BF16, tag="et")
            et_view = et_d.ap().rearrange("(j p) l -> p j l", p=P)
            for j in range(NF):
                half = L // 2
                nc.sync.dma_start(et_sb[:, j, :half], et_view[:, j, :half])
                nc.sync.dma_start(et_sb[:, j, half:], et_view[:, j, half:])

            def _cp(i, out_ap, in_ap):
                if i % 2 == 0:
                    nc.vector.tensor_copy(out_ap, in_ap)
                else:
                    nc.scalar.copy(out_ap, in_ap)

            for h in range(NH_PER_CORE):
                # ---- q^T / k^T projections: [384, 920] = W_chunk.T @ X^T ----
                qT_sb = qkt_pool.tile([P, NF, S], BF16, tag="qT")
                kT_sb = qkt_pool.tile([P, NF, S], BF16, tag="kT")
                for wi, (wname, dst) in enumerate((("wq", qT_sb), ("wk", kT_sb))):
                    w_sb = w_tiles[h][wname]
                    for m in range(NF):
                        ps0 = pmm.tile([P, NQK], F32, tag="pmm")
                        ps1 = pmm.tile([P, NQK], F32, tag="pmm")
                        for kd in range(ND):
                            wch = w_sb[:, kd, m * P : (m + 1) * P]
                            nc.tensor.matmul(
                                ps0[:], wch, xt_sb[:, kd, :NQK],
                                start=(kd == 0), stop=(kd == ND - 1),
                            )
                            nc.tensor.matmul(
                                ps1[:], wch, xt_sb[:, kd, NQK:],
                                start=(kd == 0), stop=(kd == ND - 1),
                            )
                        if wi == 0:
                            nc.scalar.copy(dst[:, m, :NQK], ps0[:])
                            nc.scalar.copy(dst[:, m, NQK:], ps1[:])
                        else:
                            nc.vector.tensor_copy(dst[:, m, :NQK], ps0[:])
                            nc.vector.tensor_copy(dst[:, m, NQK:], ps1[:])

                # ---- v projection (natural layout): [920, 384] ----
                wv_sb = w_tiles[h]["wv"]
                v_sb = v_pool.tile([P, NS, HD], BF16, tag="v")
                for c in range(NS):
                    pc = _pc(c)
                    ps = pv.tile([P, HD], F32, tag="pv")
                    for kd in range(ND):
                        nc.tensor.matmul(
                            ps[:pc, :], xt_sb[:, kd, c * P : c * P + pc],
                            wv_sb[:, kd, :],
                            start=(kd == 0), stop=(kd == ND - 1),
                        )
                    nc.scalar.copy(v_sb[:pc, c, :], ps[:pc, :])

                # ---- D = q E^T into DRAM scratch; prefetch skewed rel reads ----
                d_dram = dram_pool.tile([S, L], BF16, tag="dscratch")
                d_flat = d_dram.rearrange("a b -> (a b)")
                rel_tiles = {}
                sc_tiles = {}
                den = small_pool.tile([P, NS], F32, tag=f"den{h}")
                rden = small_pool.tile([P, NS], F32, tag=f"rden{h}")
                ncp = [0]

                def emit_d(c):
                    pc = _pc(c)
                    i_max = c * P + pc - 1
                    l_lo = (M - 1) - i_max
                    l_hi = (L - 1) - c * P + 1
                    width = l_hi - l_lo
                    nt = 3
                    base = width // nt
                    sizes = [base + (1 if i < width % nt else 0) for i in range(nt)]
                    off = l_lo
                    for w in sizes:
                        ps = pmm.tile([P, NQK], F32, tag="pmm", name="psd")
                        for kd in range(NF):
                            nc.tensor.matmul(
                                ps[:pc, :w],
                                qT_sb[:, kd, c * P : c * P + pc],
                                et_sb[:, kd, off : off + w],
                                start=(kd == 0), stop=(kd == NF - 1),
                            )
                        dstg = dst_pool.tile([P, NQK], BF16, tag="dstg", name="dstg")
                        _cp(ncp[0], dstg[:pc, :w], ps[:pc, :w])
                        ncp[0] += 1
                        nc.sync.dma_start(
                            d_dram[c * P : c * P + pc, off : off + w], dstg[:pc, :w]
                        )
                        off += w
                    # prefetch this chunk's rotated rel rows (skewed view)
                    rel_sb = rel_pool.tile([P, S], BF16, tag="rel", name="rel")
                    skew = (
                        d_flat[
                            (M - 1) + c * P * (L - 1) :
                            (M - 1) + c * P * (L - 1) + pc * (L - 1)
                        ]
                        .rearrange("(p x) -> p x", x=L - 1)
                    )
                    nc.sync.dma_start(rel_sb[:pc, :NQK], skew[:, :NQK])
                    nc.sync.dma_start(rel_sb[:pc, NQK:S], skew[:, NQK:S])
                    rel_tiles[c] = rel_sb

                def emit_scores(c):
                    pc = _pc(c)
                    rel_sb = rel_tiles[c]
                    sc32 = sc32_pool.tile([P, S], F32, tag="sc32", name="sc32")
                    sc_sb = sc_pool.tile([P, S], BF16, tag="sc", name="sc")
                    for n in range(2):
                        ps = pmm.tile([P, NQK], F32, tag="pmm", name="pss")
                        for kd in range(NF):
                            nc.tensor.matmul(
                                ps[:pc, :],
                                qT_sb[:, kd, c * P : c * P + pc],
                                kT_sb[:, kd, n * NQK : (n + 1) * NQK],
                                start=(kd == 0), stop=(kd == NF - 1),
                            )
                        nc.vector.tensor_add(
                            sc32[:pc, n * NQK : (n + 1) * NQK],
                            ps[:pc, :],
                            rel_sb[:pc, n * NQK : (n + 1) * NQK],
                        )
                    nc.scalar.activation(
                        sc_sb[:pc, :],
                        sc32[:pc, :],
                        mybir.ActivationFunctionType.Exp,
                        scale=float(1.0 / math.sqrt(HD)),
                        accum_out=den[:pc, c : c + 1],
                    )
                    nc.vector.reciprocal(rden[:pc, c : c + 1], den[:pc, c : c + 1])
                    sc_tiles[c] = sc_sb

                def emit_tctx(c):
                    pc = _pc(c)
                    sc_sb = sc_tiles[c]
                    # transpose exp(scores) chunk -> pT [k-part, q]
                    pT_sb = pT_pool.tile([P, NS, P], BF16, tag="pT", name="pT")
                    for kc in range(NS):
                        pkc = _pc(kc)
                        psT = pt.tile([P, P], BF16, tag="pt", name="psT")
                        nc.tensor.transpose(
                            psT[:pkc, :pc],
                            sc_sb[:pc, kc * P : kc * P + pkc],
                            ident[:pc, :pc],
                        )
                        _cp(kc, pT_sb[:pkc, kc, :pc], psT[:pkc, :pc])

                    # ctx = probsT.T @ v for this q-chunk
                    psc = pv.tile([P, HD], F32, tag="pv", name="psc")
                    for kc in range(NS):
                        pkc = _pc(kc)
                        nc.tensor.matmul(
                            psc[:pc, :],
                            pT_sb[:pkc, kc, :pc],
                            v_sb[:pkc, kc, :],
                            start=(kc == 0), stop=(kc == NS - 1),
                        )
                    o_sb = out_pool.tile([P, HD], F32, tag="o", name="o")
                    nc.scalar.activation(
                        o_sb[:pc, :],
                        psc[:pc, :],
                        mybir.ActivationFunctionType.Copy,
                        scale=rden[:pc, c : c + 1],
                    )
                    nc.sync.dma_start(
                        out_d.ap()[h, c * P : c * P + pc, :], o_sb[:pc, :]
                    )

                # software-pipelined: D runs 4 chunks ahead of scores; the
                # transpose+ctx of chunk i-1 fills the exp latency of chunk i
                DAHEAD = 4
                for i in range(DAHEAD):
                    emit_d(CHUNKS[i])
                for i, c in enumerate(CHUNKS):
                    emit_scores(c)
                    if i + DAHEAD < NS:
                        emit_d(CHUNKS[i + DAHEAD])
                    if i > 0:
                        emit_tctx(CHUNKS[i - 1])
                emit_tctx(CHUNKS[-1])

    nc.compile()
    return nc


_NC = None
LAST_RESULTS = None


def kernel(hidden_states, q_w, k_w, v_w, dist_emb):
    global _NC, LAST_RESULTS
    if _NC is None:
        _NC = build_kernel()

    hidden_states = np.asarray(hidden_states, dtype=np.float32)
    q_w = np.asarray(q_w, dtype=np.float32)
    k_w = np.asarray(k_w, dtype=np.float32)
    v_w = np.asarray(v_w, dtype=np.float32)
    dist_emb = np.asarray(dist_emb, dtype=np.float32)

    et = np.ascontiguousarray(dist_emb.T).astype(NPBF16)
    in_maps = []
    for core in range(8):
        b, hp = core // 2, core % 2
        sl = slice(hp * NH_PER_CORE * HD, (hp + 1) * NH_PER_CORE * HD)
        in_maps.append(
            {
                "xt": np.ascontiguousarray(hidden_states[b].T).astype(NPBF16),
                "wq": np.ascontiguousarray(q_w[:, sl]).astype(NPBF16),
                "wk": np.ascontiguousarray(k_w[:, sl]).astype(NPBF16),
                "wv": np.ascontiguousarray(v_w[:, sl]).astype(NPBF16),
                "et": et,
            }
        )

    res = run_bass_kernel_spmd(_NC, in_maps, core_ids=list(range(8)))
    LAST_RESULTS = res

    B = hidden_states.shape[0]
    out = np.empty((B, S, 4 * HD), np.float32)
    for core in range(8):
        b, hp = core // 2, core % 2
        o = res.results[core]["out"]
        for j in range(NH_PER_CORE):
            h = hp * NH_PER_CORE + j
            out[b, :, h * HD : (h + 1) * HD] = o[j]
    return out
